# revision 39
# baseline (speedup 1.0000x reference)
"""DetectionLoss Bass kernel for Trainium2 (8 NeuronCores, data-parallel over batch).

Reference computation (per level l with HW_l anchor points):
  d2[b,n,j] = ||gt_xy[b,n] - pred_xy[b,j]||^2          (n<128 gts, j<HW_l)
  match = argmin_j d2 ; valid = min_d2 < 6.25
  ce    = cross_entropy(cls[b, match], label[b,n])
  l1    = |reg[b, match] - gt_box[b,n]|.sum()
  cls_loss = sum(ce*valid); reg_loss = sum(l1*valid); num_pos = sum(valid)
Outputs: (cls_loss/max(num_pos,1), reg_loss/max(num_pos,1), num_pos)

The wall/exec time of this problem is dominated by host->device transfer over
the axon tunnel (~50-85 MB/s, ~85 ms launch round-trip), not by compute, so
the kernel runs as two small-input phases instead of shipping the 172 MB
pred_cls tensors to the devices (~9 MB total shipped, ~25x less):

Phase 1 (assignment): ships only the raw fp32 prediction xy coordinates
([BPC, 2, HWT], ~0.5 MB/core) + bf16 triple-split gt coords. On device, DVE
splits each coordinate into three bf16 components (casts + exact residual
subtractions; 8+8+8 mantissa bits >= 24 so h+m+lo reconstructs the fp32 value
exactly). PE then materializes dx=px-gx / dy=py-gy bit-exactly via an
accumulating matmul chain (ones x h/m/lo, then (-g-splits) x ones; every PSUM
partial sum is exactly representable, the final add is a Sterbenz-exact
subtraction). ACT squares, DVE adds + chunk-min, a DVE is_equal*iota scan
recovers the reference argmin exactly. Output: [128, NU] packed
2*argmin_row+valid per core.

Host: gathers the 128 matched cls/reg rows per (batch, level) from the full
input arrays (pure indexing, ~1.2 MB total; cls rows cast to bf16 - measured
1.8e-6 effect on cls_loss; reg rows stay fp32 since bf16 would bias the tiny
near-duplicate L1 terms by ~0.9%).

Phase 2 (loss): ships gathered rows + gt boxes + labels + valid masks; ACT/DVE
do the softmax CE, L1 and masked partial sums, reduced on device to [128, 3]
partials per core; host sums the 8 partial triples and normalizes.

Batches are sharded 2-per-core across the 8 cores in both phases. Each phase
runs through a cached jax.jit(shard_map) wrapper (_CachedRunner) - rebuilding
it per call (as bass_utils.run_bass_kernel_spmd does) re-traces and re-lowers
the NEFF wrapper, costing ~0.2 s per launch.
"""

import numpy as np
import sys

sys.path.insert(0, "/opt/trn_rl_repo")

B, N, NC = 16, 128, 80
HWS = [25600, 6400, 1600]
HWT = sum(HWS)  # 33600
NCORES = 8
BPC = B // NCORES  # batches per core = 2
CHUNK = 1024  # points per chunk; PSUM tile [128, 2*CHUNK] = 4 banks, 2 bufs = 8
DIST2 = 6.25  # DIST_THRESH**2
NU = BPC * 3  # units (batch, level) per core = 6

lvl_off = [0, HWS[0], HWS[0] + HWS[1]]


def _chunks(hw):
    out = []
    off = 0
    while off < hw:
        w = min(CHUNK, hw - off)
        out.append((off, w))
        off += w
    return out


def build_p1():
    """Phase 1: per-gt argmin index + validity from xy coords only."""
    import concourse.bacc as bacc
    import concourse.mybir as mybir
    import concourse.tile as tile

    fp32 = mybir.dt.float32
    int32 = mybir.dt.int32
    bf16 = mybir.dt.bfloat16
    Alu = mybir.AluOpType
    Act = mybir.ActivationFunctionType
    Axis = mybir.AxisListType

    nc = bacc.Bacc("TRN2", target_bir_lowering=False, debug=False, num_devices=NCORES)

    # pmat rows: [px | py] raw fp32; the bf16 triple-split happens on device
    # (casts + exact residual subtractions; 8+8+8 mantissa bits >= 24 so the
    # reconstruction and hence dx == fp32(px-gx) stays bit-exact)
    pmat = nc.declare_dram_parameter("pmat", [BPC, 2, HWT], fp32, isOutput=False)
    # glhs[b, 0] = rows [-gxh -gxm -gxl]; glhs[b, 1] = [-gyh -gym -gyl]
    glhs = nc.declare_dram_parameter("glhs", [BPC, 2, 3, N], bf16, isOutput=False)
    # out col u in 0..5: 2*(level-local argmin row) + valid (both exact in fp32)
    out1 = nc.declare_dram_parameter("out1", [128, NU], fp32, isOutput=True)

    with tile.TileContext(nc) as tc:
        with (
            tc.tile_pool(name="const", bufs=1) as constp,
            tc.tile_pool(name="pm", bufs=4) as pmp,
            tc.tile_pool(name="psum", bufs=2, space="PSUM") as psump,
            tc.tile_pool(name="sqp", bufs=2) as sqp,
            tc.tile_pool(name="d2p", bufs=3) as d2p,
            tc.tile_pool(name="junk", bufs=2) as junkp,
            tc.tile_pool(name="perb", bufs=2) as perbp,
            tc.tile_pool(name="small", bufs=8) as smallp,
            tc.tile_pool(name="acc", bufs=1) as accp,
        ):
            # ---- constants ----
            iota_i = constp.tile([128, CHUNK], int32, tag="iota_i")
            nc.gpsimd.iota(iota_i[:], pattern=[[1, CHUNK]], base=0, channel_multiplier=0)
            iota_f = constp.tile([128, CHUNK], fp32, tag="iota_f")
            nc.vector.tensor_copy(iota_f[:], iota_i[:])

            iotac_i = constp.tile([128, 32], int32, tag="iotac_i")
            nc.gpsimd.iota(iotac_i[:], pattern=[[1, 32]], base=0, channel_multiplier=0)
            iotac_f = constp.tile([128, 32], fp32, tag="iotac_f")
            nc.vector.tensor_copy(iotac_f[:], iotac_i[:])

            # ones for the matmuls: lhsT side (px/py sum) and rhs side (-g bcast)
            ones_lhs = constp.tile([38, N], bf16, tag="ones_lhs")
            nc.vector.memset(ones_lhs[:], 1.0)
            ones_rhs = constp.tile([38, 512], bf16, tag="ones_rhs")
            nc.vector.memset(ones_rhs[:], 1.0)

            acc = accp.tile([128, NU], fp32, tag="acc")
            nc.vector.memset(acc[:], 0.0)

            for b in range(BPC):
                gl = perbp.tile([38, N], bf16, tag="gl")
                nc.scalar.dma_start(out=gl[0:3, :], in_=glhs[b, 0])
                nc.scalar.dma_start(out=gl[32:35, :], in_=glhs[b, 1])

                for l in range(3):
                    hw = HWS[l]
                    cks = _chunks(hw)
                    C = len(cks)
                    u = b * 3 + l

                    cm = smallp.tile([128, 32], fp32, tag="cm")
                    jl = smallp.tile([128, 32], fp32, tag="jl")

                    # ---- per chunk: d2 (exact diff form) -> chunk min + local argmin
                    for k, (off, w) in enumerate(cks):
                        # load px -> partition 0, py -> partition 32 (fp32)
                        xt = pmp.tile([38, CHUNK], fp32, tag="xt")
                        nc.sync.dma_start(
                            out=xt[0:1, :w],
                            in_=pmat[b, 0:1, lvl_off[l] + off : lvl_off[l] + off + w],
                        )
                        nc.sync.dma_start(
                            out=xt[32:33, :w],
                            in_=pmat[b, 1:2, lvl_off[l] + off : lvl_off[l] + off + w],
                        )
                        # triple-split on device: h=bf16(x); r=x-h; m=bf16(r);
                        # r2=r-m; lo=bf16(r2). Each component lives on partition
                        # 0 (x) / 32 (y) of its own tile so every matmul below
                        # has a legal base partition.
                        ht = pmp.tile([38, CHUNK], bf16, tag="ht")
                        rt = pmp.tile([38, CHUNK], fp32, tag="rt")
                        mt = pmp.tile([38, CHUNK], bf16, tag="mt")
                        r2t = pmp.tile([38, CHUNK], fp32, tag="r2t")
                        lt = pmp.tile([38, CHUNK], bf16, tag="lt")
                        for base in (0, 32):
                            s = slice(base, base + 1)
                            nc.vector.tensor_copy(ht[s, :w], xt[s, :w])
                            nc.vector.tensor_tensor(
                                out=rt[s, :w], in0=xt[s, :w], in1=ht[s, :w],
                                op=Alu.subtract,
                            )
                            nc.vector.tensor_copy(mt[s, :w], rt[s, :w])
                            nc.vector.tensor_tensor(
                                out=r2t[s, :w], in0=rt[s, :w], in1=mt[s, :w],
                                op=Alu.subtract,
                            )
                            nc.vector.tensor_copy(lt[s, :w], r2t[s, :w])
                        ps = psump.tile([128, 2 * CHUNK], fp32, tag="ps")
                        for so in range(0, w, 512):
                            sw = min(512, w - so)
                            # dx = ((pxh+pxm)+pxl) + (-gxh-gxm-gxl): every PSUM
                            # partial is exactly representable, final add is
                            # Sterbenz-exact -> dx == fp32(px-gx) bit-exact
                            for ci, (base, pso) in enumerate(((0, 0), (32, CHUNK))):
                                for co, comp in enumerate((ht, mt, lt)):
                                    nc.tensor.matmul(
                                        out=ps[:, pso + so : pso + so + sw],
                                        lhsT=ones_lhs[base : base + 1, :],
                                        rhs=comp[base : base + 1, so : so + sw],
                                        start=(co == 0),
                                        stop=False,
                                    )
                                nc.tensor.matmul(
                                    out=ps[:, pso + so : pso + so + sw],
                                    lhsT=gl[base : base + 3, :],
                                    rhs=ones_rhs[base : base + 3, 0:sw],
                                    start=False,
                                    stop=True,
                                )
                        sq = sqp.tile([128, 2 * CHUNK], fp32, tag="sq")
                        if w == CHUNK:
                            nc.scalar.activation(
                                out=sq[:], in_=ps[:], func=Act.Square, scale=1.0
                            )
                        else:
                            nc.scalar.activation(
                                out=sq[:, :w], in_=ps[:, :w], func=Act.Square, scale=1.0
                            )
                            nc.scalar.activation(
                                out=sq[:, CHUNK : CHUNK + w],
                                in_=ps[:, CHUNK : CHUNK + w],
                                func=Act.Square,
                                scale=1.0,
                            )
                        d2t = d2p.tile([128, CHUNK], fp32, tag="d2t")
                        nc.vector.tensor_tensor(
                            out=d2t[:, :w],
                            in0=sq[:, :w],
                            in1=sq[:, CHUNK : CHUNK + w],
                            op=Alu.add,
                        )
                        nc.vector.tensor_reduce(
                            out=cm[:, k : k + 1], in_=d2t[:, :w], axis=Axis.X, op=Alu.min
                        )
                        junk = junkp.tile([128, CHUNK], fp32, tag="junkv")
                        nc.vector.scalar_tensor_tensor(
                            out=junk[:, :w],
                            in0=d2t[:, :w],
                            scalar=cm[:, k : k + 1],
                            in1=iota_f[:, :w],
                            op0=Alu.is_equal,
                            op1=Alu.mult,
                            accum_out=jl[:, k : k + 1],
                        )

                    # ---- level decode: lvlmin, winning chunk, level-local row ----
                    lvlmin = smallp.tile([128, 1], fp32, tag="lvlmin")
                    nc.vector.tensor_reduce(
                        out=lvlmin[:], in_=cm[:, :C], axis=Axis.X, op=Alu.min
                    )
                    eqc = smallp.tile([128, 32], fp32, tag="eqc")
                    nc.vector.tensor_scalar(
                        out=eqc[:, :C],
                        in0=cm[:, :C],
                        scalar1=lvlmin[:, 0:1],
                        scalar2=None,
                        op0=Alu.is_equal,
                    )
                    junkc = smallp.tile([128, 32], fp32, tag="junkc")
                    cbase = smallp.tile([128, 1], fp32, tag="cbase")
                    nc.vector.scalar_tensor_tensor(
                        out=junkc[:, :C],
                        in0=eqc[:, :C],
                        scalar=float(CHUNK),
                        in1=iotac_f[:, :C],
                        op0=Alu.mult,
                        op1=Alu.mult,
                        accum_out=cbase[:],
                    )
                    junkc2 = smallp.tile([128, 32], fp32, tag="junkc2")
                    jloc = smallp.tile([128, 1], fp32, tag="jloc")
                    nc.vector.scalar_tensor_tensor(
                        out=junkc2[:, :C],
                        in0=jl[:, :C],
                        scalar=1.0,
                        in1=eqc[:, :C],
                        op0=Alu.mult,
                        op1=Alu.mult,
                        accum_out=jloc[:],
                    )
                    jrow_f = smallp.tile([128, 1], fp32, tag="jrow_f")
                    nc.vector.tensor_tensor(
                        out=jrow_f[:], in0=cbase[:], in1=jloc[:], op=Alu.add
                    )
                    # clamp (tie-safety): level-local row in [0, hw-1]
                    nc.vector.tensor_scalar(
                        out=jrow_f[:],
                        in0=jrow_f[:],
                        scalar1=float(hw - 1),
                        scalar2=None,
                        op0=Alu.min,
                    )
                    # valid mask: d2 < 6.25
                    wcol = smallp.tile([128, 1], fp32, tag="wcol")
                    nc.vector.tensor_scalar(
                        out=wcol[:],
                        in0=lvlmin[:],
                        scalar1=DIST2,
                        scalar2=None,
                        op0=Alu.is_lt,
                    )
                    # pack 2*jrow + valid (max 2*25599+1 < 2^24, exact in fp32)
                    nc.vector.scalar_tensor_tensor(
                        out=acc[:, u : u + 1],
                        in0=jrow_f[:],
                        scalar=2.0,
                        in1=wcol[:],
                        op0=Alu.mult,
                        op1=Alu.add,
                    )

            nc.scalar.dma_start(out=out1[:], in_=acc[:])

    nc.compile()
    return nc


def build_p2():
    """Phase 2: CE + L1 + masked partial sums from host-gathered rows."""
    import concourse.bacc as bacc
    import concourse.mybir as mybir
    import concourse.tile as tile

    fp32 = mybir.dt.float32
    int32 = mybir.dt.int32
    bf16 = mybir.dt.bfloat16
    Alu = mybir.AluOpType
    Act = mybir.ActivationFunctionType
    Axis = mybir.AxisListType

    nc = bacc.Bacc("TRN2", target_bir_lowering=False, debug=False, num_devices=NCORES)

    gcls = nc.declare_dram_parameter("gcls", [NU * N, NC], bf16, isOutput=False)
    # greg stays fp32: bf16 rounding of the gathered reg rows biases the L1
    # terms upward by ~0.9% of reg_loss (quantization grid ~ the near-duplicate
    # coordinate differences) -- measured 8.7e-3 rel, half the tolerance.
    greg = nc.declare_dram_parameter("greg", [NU * N, 4], fp32, isOutput=False)
    gtb = nc.declare_dram_parameter("gtb", [BPC, N, 4], fp32, isOutput=False)
    labf = nc.declare_dram_parameter("labf", [BPC, N, 1], fp32, isOutput=False)
    wc = nc.declare_dram_parameter("wc", [128, NU], fp32, isOutput=False)
    partials = nc.declare_dram_parameter("partials", [128, 3], fp32, isOutput=True)

    with tile.TileContext(nc) as tc:
        with (
            tc.tile_pool(name="const", bufs=1) as constp,
            tc.tile_pool(name="perb", bufs=2) as perbp,
            tc.tile_pool(name="small", bufs=8) as smallp,
            tc.tile_pool(name="acc", bufs=1) as accp,
        ):
            iota80_i = constp.tile([128, NC], int32, tag="iota80_i")
            nc.gpsimd.iota(iota80_i[:], pattern=[[1, NC]], base=0, channel_multiplier=0)
            iota80_f = constp.tile([128, NC], fp32, tag="iota80_f")
            nc.vector.tensor_copy(iota80_f[:], iota80_i[:])

            acc = accp.tile([128, 3 * NU], fp32, tag="acc")
            nc.vector.memset(acc[:], 0.0)

            wct = constp.tile([128, NU], fp32, tag="wct")
            nc.sync.dma_start(out=wct[:], in_=wc[:])

            for b in range(BPC):
                gb = perbp.tile([N, 4], fp32, tag="gb")
                nc.sync.dma_start(out=gb[:], in_=gtb[b])
                labt = perbp.tile([N, 1], fp32, tag="labt")
                nc.sync.dma_start(out=labt[:], in_=labf[b])
                oh = perbp.tile([N, NC], fp32, tag="oh")
                nc.vector.tensor_scalar(
                    out=oh[:],
                    in0=iota80_f[:],
                    scalar1=labt[:, 0:1],
                    scalar2=None,
                    op0=Alu.is_equal,
                )

                for l in range(3):
                    u = b * 3 + l
                    gctb = smallp.tile([N, NC], bf16, tag="gctb")
                    nc.sync.dma_start(out=gctb[:], in_=gcls[u * N : (u + 1) * N])
                    gct = smallp.tile([N, NC], fp32, tag="gct")
                    nc.vector.tensor_copy(gct[:], gctb[:])
                    grt = smallp.tile([N, 4], fp32, tag="grt")
                    nc.sync.dma_start(out=grt[:], in_=greg[u * N : (u + 1) * N])

                    # ---- CE ----
                    mx = smallp.tile([128, 1], fp32, tag="mx")
                    nc.vector.tensor_reduce(
                        out=mx[:], in_=gct[:], axis=Axis.X, op=Alu.max
                    )
                    nmx = smallp.tile([128, 1], fp32, tag="nmx")
                    nc.vector.tensor_scalar(
                        out=nmx[:], in0=mx[:], scalar1=-1.0, scalar2=None, op0=Alu.mult
                    )
                    expt = smallp.tile([N, NC], fp32, tag="expt")
                    se = smallp.tile([128, 1], fp32, tag="se")
                    nc.scalar.activation(
                        out=expt[:],
                        in_=gct[:],
                        func=Act.Exp,
                        bias=nmx[:, 0:1],
                        scale=1.0,
                        accum_out=se[:],
                    )
                    lse = smallp.tile([128, 1], fp32, tag="lse")
                    nc.scalar.activation(out=lse[:], in_=se[:], func=Act.Ln)
                    junk80 = smallp.tile([N, NC], fp32, tag="junk80")
                    pk = smallp.tile([128, 1], fp32, tag="pk")
                    nc.vector.scalar_tensor_tensor(
                        out=junk80[:],
                        in0=gct[:],
                        scalar=1.0,
                        in1=oh[:],
                        op0=Alu.mult,
                        op1=Alu.mult,
                        accum_out=pk[:],
                    )
                    ce = smallp.tile([128, 1], fp32, tag="ce")
                    nc.vector.tensor_tensor(out=ce[:], in0=mx[:], in1=lse[:], op=Alu.add)
                    nc.vector.tensor_tensor(
                        out=ce[:], in0=ce[:], in1=pk[:], op=Alu.subtract
                    )

                    # ---- L1 ----
                    df = smallp.tile([N, 4], fp32, tag="df")
                    nc.vector.tensor_tensor(
                        out=df[:], in0=grt[:], in1=gb[:], op=Alu.subtract
                    )
                    l1 = smallp.tile([128, 1], fp32, tag="l1")
                    nc.vector.tensor_reduce(
                        out=l1[:],
                        in_=df[:],
                        axis=Axis.X,
                        op=Alu.add,
                        apply_absolute_value=True,
                    )

                    # ---- masked accumulate ----
                    nc.vector.tensor_tensor(
                        out=acc[:, 0 * NU + u : 0 * NU + u + 1],
                        in0=ce[:],
                        in1=wct[:, u : u + 1],
                        op=Alu.mult,
                    )
                    nc.vector.tensor_tensor(
                        out=acc[:, 1 * NU + u : 1 * NU + u + 1],
                        in0=l1[:],
                        in1=wct[:, u : u + 1],
                        op=Alu.mult,
                    )
                    nc.vector.tensor_copy(
                        acc[:, 2 * NU + u : 2 * NU + u + 1], wct[:, u : u + 1]
                    )

            # per-core reduce over the 6 units -> [128, 3]
            accr = accp.tile([128, 3], fp32, tag="accr")
            for comp in range(3):
                nc.vector.tensor_reduce(
                    out=accr[:, comp : comp + 1],
                    in_=acc[:, comp * NU : (comp + 1) * NU],
                    axis=Axis.X,
                    op=Alu.add,
                )
            nc.scalar.dma_start(out=partials[:], in_=accr[:])

    nc.compile()
    return nc


class _CachedRunner:
    """Build the jitted shard_map wrapper for a compiled Bass module ONCE and
    reuse it across calls.

    bass_utils.run_bass_kernel_spmd -> bass2jax.run_bass_via_pjrt constructs a
    fresh closure + jax.jit object per invocation, so every call re-traces and
    re-lowers the NEFF wrapper (~0.2 s of walrus/DVE-table work). Caching the
    jitted callable removes that fixed cost; only input upload + execute +
    output fetch remain. Takes/returns GLOBAL arrays ([n_cores*dim0, ...]),
    avoiding per-core slicing copies.
    """

    def __init__(self, nc, n_cores=NCORES):
        import jax
        import concourse.mybir as mybir
        from jax.sharding import Mesh, PartitionSpec
        from jax.experimental.shard_map import shard_map
        from concourse.bass2jax import (
            _bass_exec_p,
            install_neuronx_cc_hook,
            partition_id_tensor,
        )

        install_neuronx_cc_hook()
        assert nc.dbg_addr is None
        self.n_cores = n_cores

        partition_name = (
            nc.partition_id_tensor.name if nc.partition_id_tensor else None
        )
        in_names, out_names, out_shapes, out_dtypes, out_avals = [], [], [], [], []
        for alloc in nc.m.functions[0].allocations:
            if not isinstance(alloc, mybir.MemoryLocationSet):
                continue
            name = alloc.memorylocations[0].name
            if alloc.kind == "ExternalInput":
                if name != partition_name:
                    in_names.append(name)
            elif alloc.kind == "ExternalOutput":
                shape = tuple(alloc.tensor_shape)
                dtype = mybir.dt.np(alloc.dtype)
                out_names.append(name)
                out_shapes.append(shape)
                out_dtypes.append(dtype)
                out_avals.append(jax.core.ShapedArray(shape, dtype))
        self.in_names = list(in_names)
        self.out_names = out_names
        self.out_shapes = out_shapes
        self.out_dtypes = out_dtypes
        n_params = len(in_names)
        n_outs = len(out_names)
        all_in_names = in_names + out_names
        if partition_name is not None:
            all_in_names.append(partition_name)

        def _body(*args):
            operands = list(args)
            if partition_name is not None:
                operands.append(partition_id_tensor())
            outs = _bass_exec_p.bind(
                *operands,
                out_avals=tuple(out_avals),
                in_names=tuple(all_in_names),
                out_names=tuple(out_names),
                lowering_input_output_aliases=(),
                sim_require_finite=True,
                sim_require_nnan=True,
                nc=nc,
            )
            return tuple(outs)

        devices = jax.devices()[:n_cores]
        assert len(devices) == n_cores
        mesh = Mesh(np.asarray(devices), ("core",))
        in_specs = (PartitionSpec("core"),) * (n_params + n_outs)
        out_specs = (PartitionSpec("core"),) * n_outs
        self._fn = jax.jit(
            shard_map(
                _body, mesh=mesh, in_specs=in_specs, out_specs=out_specs,
                check_rep=False,
            ),
            donate_argnums=tuple(range(n_params, n_params + n_outs)),
            keep_unused=True,
        )
        # async device_put of inputs before dispatch skips a synchronous host
        # staging copy inside the jit call (~10 ms on a 4 MB payload)
        from jax.sharding import NamedSharding

        self._jdp = jax.device_put
        self._in_sh = NamedSharding(mesh, PartitionSpec("core"))
        self._dp_ok = True

    def put(self, arr):
        """Start an async upload of one input early (overlaps later host prep)."""
        if not self._dp_ok:
            return arr
        try:
            return self._jdp(np.ascontiguousarray(arr), self._in_sh)
        except Exception:
            return arr

    def __call__(self, global_ins):
        """global_ins: dict name -> [n_cores*dim0, ...] array (numpy, or a
        jax array already uploaded via put()). Returns dict name ->
        [n_cores, *shape] array."""
        args = [
            v if not isinstance(v, np.ndarray) else np.ascontiguousarray(v)
            for v in (global_ins[n] for n in self.in_names)
        ]
        if self._dp_ok:
            try:
                args = [
                    a if not isinstance(a, np.ndarray) else self._jdp(a, self._in_sh)
                    for a in args
                ]
            except Exception:
                self._dp_ok = False
                args = [np.ascontiguousarray(global_ins[n]) for n in self.in_names]
        zeros = [
            np.zeros((self.n_cores * s[0], *s[1:]), d)
            for s, d in zip(self.out_shapes, self.out_dtypes)
        ]
        outs = self._fn(*args, *zeros)
        return {
            name: np.asarray(outs[i]).reshape(self.n_cores, *self.out_shapes[i])
            for i, name in enumerate(self.out_names)
        }


def _split3(x):
    """Exact fp32 -> 3x bf16 split: hi+mid+lo reconstructs x to < 0.5 ulp."""
    import ml_dtypes

    bf16 = ml_dtypes.bfloat16
    h = x.astype(bf16)
    r = x - h.astype(np.float32)
    m = r.astype(bf16)
    lo = (r - m.astype(np.float32)).astype(bf16)
    return h, m, lo


def host_prep1(pred_reg, gt_boxes, put=None):
    """Global phase-1 inputs: raw fp32 xy (split happens on device) + gt splits.

    If `put` is given, the 4.3 MB pmat upload is started asynchronously as soon
    as it is built, overlapping the gt-split prep below.
    """
    import ml_dtypes

    bf16 = ml_dtypes.bfloat16

    pmat_all = np.empty((B, 2, HWT), dtype=np.float32)
    off = 0
    for l, hw in enumerate(HWS):
        pmat_all[:, 0, off : off + hw] = pred_reg[l][:, :, 0]
        pmat_all[:, 1, off : off + hw] = pred_reg[l][:, :, 1]
        off += hw
    if put is not None:
        pmat_all = put(pmat_all)

    g = gt_boxes[:, :, :2].astype(np.float32)
    gxh, gxm, gxl = _split3(np.ascontiguousarray(g[:, :, 0]))
    gyh, gym, gyl = _split3(np.ascontiguousarray(g[:, :, 1]))
    glhs_all = np.empty((B, 2, 3, N), dtype=bf16)
    glhs_all[:, 0, 0] = -gxh
    glhs_all[:, 0, 1] = -gxm
    glhs_all[:, 0, 2] = -gxl
    glhs_all[:, 1, 0] = -gyh
    glhs_all[:, 1, 1] = -gym
    glhs_all[:, 1, 2] = -gyl

    return {"pmat": pmat_all, "glhs": glhs_all}


def host_gather(out1, pred_cls, pred_reg, gt_boxes, gt_labels, put=None):
    """Decode phase-1 outputs, gather matched rows, build global phase-2 inputs.

    out1: [NCORES, 128, NU] fp32, col u = 2*jrow + valid. If `put` is given,
    the 1 MB gcls upload starts asynchronously before the remaining prep.
    """
    import ml_dtypes

    bf16 = ml_dtypes.bfloat16
    packed = out1.astype(np.int64)
    jrows = packed >> 1  # [NCORES, 128, NU] level-local rows
    valid = (packed & 1).astype(np.float32)

    gc = np.empty((NCORES, NU * N, NC), dtype=bf16)
    gr = np.empty((NCORES, NU * N, 4), dtype=np.float32)
    for l in range(3):
        cls_flat = pred_cls[l].reshape(B * HWS[l], NC)
        reg_flat = pred_reg[l].reshape(B * HWS[l], 4)
        for b in range(BPC):
            u = b * 3 + l
            rows = (np.arange(NCORES)[:, None] * BPC + b) * HWS[l] + jrows[:, :, u]
            gc[:, u * N : (u + 1) * N] = cls_flat[rows].astype(bf16)
            gr[:, u * N : (u + 1) * N] = reg_flat[rows]
    gcls = gc.reshape(NCORES * NU * N, NC)
    if put is not None:
        gcls = put(gcls)
    return {
        "gcls": gcls,
        "greg": gr.reshape(NCORES * NU * N, 4),
        "gtb": gt_boxes.astype(np.float32),
        "labf": gt_labels.astype(np.float32).reshape(B, N, 1),
        "wc": valid.reshape(NCORES * 128, NU),
    }


class _FallbackRunner:
    """Slow-path runner via bass_utils.run_bass_kernel_spmd (per-call jit)."""

    def __init__(self, nc):
        self.nc = nc

    def __call__(self, global_ins):
        from concourse.bass_utils import run_bass_kernel_spmd

        in_maps = []
        for c in range(NCORES):
            m = {}
            for name, arr in global_ins.items():
                d0 = arr.shape[0] // NCORES
                m[name] = np.ascontiguousarray(arr[c * d0 : (c + 1) * d0])
            in_maps.append(m)
        res = run_bass_kernel_spmd(self.nc, in_maps, list(range(NCORES)))
        out = {}
        for name in res.results[0]:
            out[name] = np.stack([np.asarray(r[name]) for r in res.results])
        return out


_NC_CACHE = {}


def _get_run1():
    if "p1" not in _NC_CACHE:
        nc = build_p1()
        try:
            _NC_CACHE["p1"] = _CachedRunner(nc)
        except Exception:
            _NC_CACHE["p1"] = _FallbackRunner(nc)
    return _NC_CACHE["p1"]


def _get_run2():
    if "p2" not in _NC_CACHE:
        nc = build_p2()
        try:
            _NC_CACHE["p2"] = _CachedRunner(nc)
        except Exception:
            _NC_CACHE["p2"] = _FallbackRunner(nc)
    return _NC_CACHE["p2"]


def kernel(
    pred_cls_0,
    pred_cls_1,
    pred_cls_2,
    pred_reg_0,
    pred_reg_1,
    pred_reg_2,
    gt_boxes,
    gt_labels,
):
    pred_cls = [np.asarray(pred_cls_0), np.asarray(pred_cls_1), np.asarray(pred_cls_2)]
    pred_reg = [np.asarray(pred_reg_0), np.asarray(pred_reg_1), np.asarray(pred_reg_2)]
    gt_boxes = np.asarray(gt_boxes)
    gt_labels = np.asarray(gt_labels)

    run1 = _get_run1()
    run2 = _get_run2()

    in1 = host_prep1(pred_reg, gt_boxes, put=getattr(run1, "put", None))
    out1 = run1(in1)["out1"].astype(np.float32)  # [NCORES, 128, NU]

    in2 = host_gather(
        out1, pred_cls, pred_reg, gt_boxes, gt_labels,
        put=getattr(run2, "put", None),
    )
    partials = run2(in2)["partials"]  # [NCORES, 128, 3]

    p = np.asarray(partials, dtype=np.float64)
    cls_loss, reg_loss, num_pos = p.sum(axis=(0, 1))
    denom = max(num_pos, 1.0)
    return (
        np.float32(cls_loss / denom),
        np.float32(reg_loss / denom),
        np.float32(num_pos),
    )
